# revision 13
# baseline (speedup 1.0000x reference)
"""Trainium2 Bass kernel for the BaselineNCDE problem.

Math (per batch row b):
    cp[t]   = [time[t], features[t]]                      (C=7)
    h0      = Wp @ features[0] + bp                       (H=64)
    U[t]    = (cp[min(t+1,T-1)] - cp[t]) * (t < length)   (C,)   [host-folded:
              equals dxdt*dt*active of the reference, exactly]
    for t in 0..T-1:
        x1 = gelu_tanh(W1 @ h + b1)                       (128,)
        x2 = gelu_tanh(W2 @ x1 + b2)                      (128,)
        M  = tanh(W3 @ x2 + b3)  reshaped (H, C)
        h  = h + M @ U[t]
        pred[t] = Wr2 @ relu(Wr1 @ h + br1) + br2

Device mapping (per core, BS=512 batch rows, 2 pipelined streams of NB=256):
  batch on the free axis, feature dims on partitions.  PE does all matmuls
  (incl. rank-1 bias injection and the c-contraction via a 0/1 pattern
  matrix), ACT does gelu/tanh, DVE does the M*U elementwise multiply
  (U replicated across partitions by a stride-0 DMA), h update, and relu.
"""

import numpy as np

B, T, F = 4096, 256, 6
H, W = 64, 128
C = F + 1            # 7
HC = H * C           # 448
NCORES = 8
BS = B // NCORES     # 512 batch rows per core
NB = 256             # stream width (batch cols per stream)
NS = 2               # pipelined streams
CH = 112             # mm3 chunk rows (448 = 4*112; 112 % 7 == 0)
NCHUNK = 4
R1 = 32              # readout hidden
EP = 128             # pred rows per epoch (sbuf pred tile partitions)

_BUILD_CACHE = {}


def _build(t_steps=T, mult_bf16=False):
    """Build the Bacc module (same program for every core). Returns (nc, names)."""
    key = (t_steps, mult_bf16)
    if key in _BUILD_CACHE:
        return _BUILD_CACHE[key]

    from contextlib import ExitStack

    import concourse.bass as bass
    import concourse.mybir as mybir
    import concourse.tile as tile
    from concourse import bacc

    dt = mybir.dt
    AF = mybir.ActivationFunctionType
    ALU = mybir.AluOpType
    f32 = dt.float32
    mdt = dt.bfloat16 if mult_bf16 else f32

    assert t_steps % 4 == 0
    nc = bacc.Bacc("TRN2", target_bir_lowering=False, debug=False)

    # ---- DRAM I/O ----
    u_d = nc.dram_tensor("u", [t_steps, C, BS], mdt, kind="ExternalInput")
    f0_d = nc.dram_tensor("f0t", [F, BS], f32, kind="ExternalInput")
    w1t_d = nc.dram_tensor("w1t", [H, W], f32, kind="ExternalInput")
    w2t_d = nc.dram_tensor("w2t", [W, W], f32, kind="ExternalInput")
    w3t_d = nc.dram_tensor("w3t", [W, HC], f32, kind="ExternalInput")
    b1_d = nc.dram_tensor("b1c", [W, 1], f32, kind="ExternalInput")
    b2_d = nc.dram_tensor("b2c", [W, 1], f32, kind="ExternalInput")
    b3_d = nc.dram_tensor("b3r", [NCHUNK, HC], f32, kind="ExternalInput")
    a_d = nc.dram_tensor("ared", [CH, NCHUNK * H], mdt, kind="ExternalInput")
    wpt_d = nc.dram_tensor("wpt", [F, H], f32, kind="ExternalInput")
    bp_d = nc.dram_tensor("bpr", [1, H], f32, kind="ExternalInput")
    wr1t_d = nc.dram_tensor("wr1t", [H, R1], f32, kind="ExternalInput")
    br1_d = nc.dram_tensor("br1c", [R1, 1], f32, kind="ExternalInput")
    wr2t_d = nc.dram_tensor("wr2t", [R1, 1], f32, kind="ExternalInput")
    br2_d = nc.dram_tensor("br2c", [4, 1], f32, kind="ExternalInput")
    pred_d = nc.dram_tensor("pred", [t_steps, BS], f32, kind="ExternalOutput")

    with tile.TileContext(nc) as tc, ExitStack() as ctx:
        const = ctx.enter_context(tc.tile_pool(name="const", bufs=1))

        def load_const(dram, shape, dtype, tag):
            t_ = const.tile(shape, dtype, tag=tag)
            nc.sync.dma_start(t_[:], dram.ap())
            return t_

        w1t_s = load_const(w1t_d, [H, W], f32, "w1t")
        w2t_s = load_const(w2t_d, [W, W], f32, "w2t")
        w3t_s = load_const(w3t_d, [W, HC], f32, "w3t")
        b1_s = load_const(b1_d, [W, 1], f32, "b1")
        b2_s = load_const(b2_d, [W, 1], f32, "b2")
        a_s = load_const(a_d, [CH, NCHUNK * H], mdt, "ared")
        wpt_s = load_const(wpt_d, [F, H], f32, "wpt")
        bp_s = load_const(bp_d, [1, H], f32, "bp")
        wr1t_s = load_const(wr1t_d, [H, R1], f32, "wr1t")
        br1_s = load_const(br1_d, [R1, 1], f32, "br1")
        wr2t_s = load_const(wr2t_d, [R1, 1], f32, "wr2t")
        br2_s = load_const(br2_d, [4, 1], f32, "br2")
        f0_s = load_const(f0_d, [F, BS], f32, "f0")
        # b3 replicated to partitions {0,32,64,96} for row-tiled rank-1 bias mms
        b3_s = const.tile([97, HC], f32, tag="b3")
        b3_pitch = b3_s[:].ap[0][0]
        nc.sync.dma_start(
            bass.AP(b3_s.tensor, b3_s.offset, [[32 * b3_pitch, NCHUNK], [1, HC]]),
            b3_d.ap(),
        )
        # all-ones rows on partitions 0..96 (only 0,32,64,96 used)
        ones_s = const.tile([97, NB], f32, tag="ones")
        nc.vector.memset(ones_s[:], 1.0)

        state = ctx.enter_context(tc.tile_pool(name="state", bufs=1))
        h_s = [
            state.tile([H, NB], f32, tag=f"h{s}", name=f"h{s}") for s in range(NS)
        ]

        u_pool = ctx.enter_context(tc.tile_pool(name="upool", bufs=3))
        x_pool = ctx.enter_context(tc.tile_pool(name="xpool", bufs=2))
        m_pool = ctx.enter_context(tc.tile_pool(name="mpool", bufs=2))
        r_pool = ctx.enter_context(tc.tile_pool(name="rpool", bufs=2))
        pred_pool = ctx.enter_context(tc.tile_pool(name="predpool", bufs=2))

        ps_y = [
            ctx.enter_context(tc.tile_pool(name=f"psy{s}", bufs=1, space="PSUM"))
            for s in range(NS)
        ]
        ps_3 = [
            ctx.enter_context(tc.tile_pool(name=f"ps3{s}", bufs=1, space="PSUM"))
            for s in range(NS)
        ]
        ps_r2 = [
            ctx.enter_context(tc.tile_pool(name=f"psr2{s}", bufs=1, space="PSUM"))
            for s in range(NS)
        ]
        r2acc = [
            ps_r2[s].tile([EP, NB], f32, tag=f"r2acc{s}", name=f"r2acc{s}")
            for s in range(NS)
        ]

        # ---- h0 = Wp @ f0 + bp ----
        for s in range(NS):
            hp = ps_y[s].tile([H, NB], f32, tag=f"y{s}")
            nc.tensor.matmul(
                hp[:], bp_s[:1, :], ones_s[:1, :], start=True, stop=False
            )
            nc.tensor.matmul(
                hp[:],
                wpt_s[:],
                f0_s[:, s * NB : (s + 1) * NB],
                start=False,
                stop=True,
            )
            nc.vector.tensor_copy(h_s[s][:], hp[:])

        # ---- main scan ----
        # pred staging: partition = t % 4, columns = (t//4)*BS + b
        n_g = t_steps // 4
        pred_sb = pred_pool.tile([4, n_g * BS], f32, tag="pred", name="pred_sb")

        for t in range(t_steps):
            g = t // 4
            for s in range(NS):
                cs = slice(s * NB, (s + 1) * NB)

                u_t = u_pool.tile([CH, NB], mdt, tag=f"u{s}")
                nc.sync.dma_start(
                    u_t[:],
                    bass.AP(u_d, (t * C) * BS + s * NB, [[0, CH // C], [BS, C], [1, NB]]),
                )

                # x1 = gelu(W1 @ h + b1)
                y1 = ps_y[s].tile([W, NB], f32, tag=f"y{s}")
                nc.tensor.matmul(y1[:], w1t_s[:], h_s[s][:], start=True, stop=True)
                x1 = x_pool.tile([W, NB], f32, tag=f"x1{s}")
                nc.scalar.activation(x1[:], y1[:], AF.Gelu_apprx_tanh, bias=b1_s[:])

                # x2 = gelu(W2 @ x1 + b2)
                y2 = ps_y[s].tile([W, NB], f32, tag=f"y{s}")
                nc.tensor.matmul(y2[:], w2t_s[:], x1[:], start=True, stop=True)
                x2 = x_pool.tile([W, NB], f32, tag=f"x2{s}")
                nc.scalar.activation(x2[:], y2[:], AF.Gelu_apprx_tanh, bias=b2_s[:])

                # Y3 = W3 @ x2 + b3   (4 chunks of 112 rows into one 2-bank psum)
                # One accumulation group per 2KB bank (= two NB-col slices):
                # rank-1 bias mms first (start marks the bank), then W3 chunks.
                y3 = ps_3[s].tile([CH, NCHUNK * NB], f32, tag=f"y3{s}")
                for bank in range(NCHUNK // 2):
                    j0, j1 = 2 * bank, 2 * bank + 1
                    for idx, j in enumerate((j0, j1)):
                        nc.tensor.matmul(
                            y3[:, j * NB : (j + 1) * NB],
                            b3_s[32 * j : 32 * j + 1, j * CH : (j + 1) * CH],
                            ones_s[32 * j : 32 * j + 1, :],
                            start=(idx == 0),
                            stop=False,
                            tile_position=(32 * j, 0),
                        )
                    for idx, j in enumerate((j0, j1)):
                        nc.tensor.matmul(
                            y3[:, j * NB : (j + 1) * NB],
                            w3t_s[:, j * CH : (j + 1) * CH],
                            x2[:],
                            start=False,
                            stop=(idx == 1),
                        )

                # M = tanh(Y3); P = M * U_bcast
                m_t = m_pool.tile([CH, NCHUNK * NB], mdt, tag=f"m{s}")
                nc.scalar.activation(m_t[:], y3[:], AF.Tanh)
                p_t = m_pool.tile([CH, NCHUNK * NB], mdt, tag=f"p{s}")
                m3 = m_t[:].rearrange("p (j n) -> p j n", j=NCHUNK)
                p3 = p_t[:].rearrange("p (j n) -> p j n", j=NCHUNK)
                u3 = bass.AP(
                    u_t.tensor,
                    u_t.offset,
                    [list(u_t.ap[0]), [0, NCHUNK], [1, NB]],
                )
                nc.vector.tensor_tensor(p3, m3, u3, op=ALU.mult)

                # dh = A^T @ P  (c-contraction), accumulate 4 chunks
                dh = ps_y[s].tile([H, NB], f32, tag=f"y{s}")
                for j in range(NCHUNK):
                    nc.tensor.matmul(
                        dh[:],
                        a_s[:, j * H : (j + 1) * H],
                        p_t[:, j * NB : (j + 1) * NB],
                        start=(j == 0),
                        stop=(j == NCHUNK - 1),
                    )

                # h += dh
                nc.vector.tensor_add(h_s[s][:], h_s[s][:], dh[:])

                # readout: r1 = Wr1 @ h; rl = relu(r1 + br1); r2 = Wr2 @ rl
                r1 = ps_y[s].tile([R1, NB], f32, tag=f"y{s}")
                nc.tensor.matmul(r1[:], wr1t_s[:], h_s[s][:], start=True, stop=True)
                rl = r_pool.tile([R1, NB], f32, tag=f"rl{s}")
                nc.vector.tensor_scalar(
                    rl[:], r1[:], br1_s[:], 0.0, op0=ALU.add, op1=ALU.max
                )
                q = t % 4
                nc.tensor.matmul(
                    r2acc[s][32 * q : 32 * q + 1, :],
                    wr2t_s[:],
                    rl[:],
                    start=True,
                    stop=True,
                    tile_position=(0, 32 * q),
                )
                if q == 3:
                    # evacuate 4 rows of r2acc (partitions 0,32,64,96) + br2
                    src = bass.AP(
                        r2acc[s].tensor,
                        r2acc[s].offset,
                        [[32 * r2acc[s].ap[0][0], 4], [1, NB]],
                    )
                    dst = pred_sb[0:4, g * BS + s * NB : g * BS + (s + 1) * NB]
                    nc.vector.tensor_scalar(dst, src, br2_s[:], None, op0=ALU.add)

        # final: pred_sb (4, G*BS) -> pred_d (T, BS), t = 4g + p
        nc.sync.dma_start(
            bass.AP(pred_d, 0, [[BS, 4], [4 * BS, n_g], [1, BS]]),
            pred_sb[:].rearrange("p (g b) -> p g b", b=BS),
        )

    nc.compile()
    names = [
        "u", "f0t", "w1t", "w2t", "w3t", "b1c", "b2c", "b3r", "ared",
        "wpt", "bpr", "wr1t", "br1c", "wr2t", "br2c",
    ]
    _BUILD_CACHE[key] = (nc, names)
    return nc, names


def _host_prep(time, features, mask, length, Wp, bp, W1, b1, W2, b2, W3, b3,
               Wr1, br1, Wr2, br2, t_steps=T, mult_bf16=False):
    """Shard + marshal inputs into per-core in_maps."""
    time = np.asarray(time, np.float32)
    features = np.asarray(features, np.float32)
    length = np.asarray(length)
    mdt = np.dtype("bfloat16") if mult_bf16 else np.float32
    if mult_bf16:
        import ml_dtypes
        mdt = ml_dtypes.bfloat16

    cp = np.concatenate([time[..., None], features], axis=-1)  # (B, Tfull, C)
    cp_next = np.concatenate([cp[:, 1:], cp[:, -1:]], axis=1)
    active = (np.arange(cp.shape[1])[None, :] < np.asarray(length)[:, None])
    u_full = (cp_next - cp) * active[..., None].astype(np.float32)  # (B, Tfull, C)
    u_full = u_full[:, :t_steps]

    # reduce pattern A: (112, 4*64), A[p, j*64+h] = 1 iff (112*j+p)//7 == h
    a_mat = np.zeros((CH, NCHUNK * H), np.float32)
    for j in range(NCHUNK):
        for p in range(CH):
            h = (CH * j + p) // C
            a_mat[p, j * H + h] = 1.0

    shared = {
        "w1t": np.ascontiguousarray(W1.T, np.float32),          # (64,128)
        "w2t": np.ascontiguousarray(W2.T, np.float32),          # (128,128)
        "w3t": np.ascontiguousarray(W3.T, np.float32),          # (128,448)
        "b1c": np.ascontiguousarray(b1.reshape(W, 1), np.float32),
        "b2c": np.ascontiguousarray(b2.reshape(W, 1), np.float32),
        "b3r": np.broadcast_to(
            np.asarray(b3, np.float32).reshape(1, HC), (NCHUNK, HC)
        ).copy(),                                               # (4,448) replicated
        "ared": a_mat.astype(mdt),
        "wpt": np.ascontiguousarray(Wp.T, np.float32),          # (6,64)
        "bpr": np.ascontiguousarray(bp.reshape(1, H), np.float32),
        "wr1t": np.ascontiguousarray(Wr1.T, np.float32),        # (64,32)
        "br1c": np.ascontiguousarray(br1.reshape(R1, 1), np.float32),
        "wr2t": np.ascontiguousarray(Wr2.T, np.float32),        # (32,1)
        "br2c": np.full((4, 1), np.float32(br2.reshape(-1)[0]), np.float32),
    }
    in_maps = []
    for i in range(NCORES):
        bsl = slice(i * BS, (i + 1) * BS)
        u_core = np.ascontiguousarray(
            u_full[bsl].transpose(1, 2, 0)
        ).astype(mdt)                                            # (T, C, BS)
        f0t = np.ascontiguousarray(features[bsl, 0, :].T, np.float32)  # (6, BS)
        m = dict(shared)
        m["u"] = u_core
        m["f0t"] = f0t
        in_maps.append(m)
    return in_maps


def kernel(**inputs):
    from concourse.bass_utils import run_bass_kernel_spmd

    t_steps = T
    mult_bf16 = False
    nc, _ = _build(t_steps=t_steps, mult_bf16=mult_bf16)
    in_maps = _host_prep(**inputs, t_steps=t_steps, mult_bf16=mult_bf16)
    res = run_bass_kernel_spmd(nc, in_maps, list(range(NCORES)))
    preds = [res.results[i]["pred"] for i in range(NCORES)]  # (T, BS) each
    out = np.concatenate([p.T for p in preds], axis=0)  # (B, T)
    return np.ascontiguousarray(out.astype(np.float32))


# revision 14
# speedup vs baseline: 4.3428x; 4.3428x over previous
"""Trainium2 Bass kernel for the BaselineNCDE problem.

Math (per batch row b):
    cp[t]   = [time[t], features[t]]                      (C=7)
    h0      = Wp @ features[0] + bp                       (H=64)
    U[t]    = (cp[min(t+1,T-1)] - cp[t]) * (t < length)   (C,)   [host-folded:
              equals dxdt*dt*active of the reference, exactly]
    for t in 0..T-1:
        x1 = gelu_tanh(W1 @ h + b1)                       (128,)
        x2 = gelu_tanh(W2 @ x1 + b2)                      (128,)
        M  = tanh(W3 @ x2 + b3)  reshaped (H, C)
        h  = h + M @ U[t]
        pred[t] = Wr2 @ relu(Wr1 @ h + br1) + br2

Device mapping (per core, BS=512 batch rows, 2 pipelined streams of NB=256):
  batch on the free axis, feature dims on partitions.  PE does all matmuls
  (incl. rank-1 bias injection and the c-contraction via a 0/1 pattern
  matrix), ACT does gelu/tanh, DVE does the M*U elementwise multiply
  (U replicated across partitions by a stride-0 DMA), h update, and relu.
"""

import numpy as np

B, T, F = 4096, 256, 6
H, W = 64, 128
C = F + 1            # 7
HC = H * C           # 448
NCORES = 8
BS = B // NCORES     # 512 batch rows per core
NB = 256             # stream width (batch cols per stream)
NS = 2               # pipelined streams
CH = 112             # mm3 chunk rows (448 = 4*112; 112 % 7 == 0)
NCHUNK = 4
R1 = 32              # readout hidden
EP = 128             # pred rows per epoch (sbuf pred tile partitions)

_BUILD_CACHE = {}


def _build(t_steps=T, mult_bf16=False):
    """Build the Bacc module (same program for every core). Returns (nc, names)."""
    key = (t_steps, mult_bf16)
    if key in _BUILD_CACHE:
        return _BUILD_CACHE[key]

    from contextlib import ExitStack

    import concourse.bass as bass
    import concourse.mybir as mybir
    import concourse.tile as tile
    from concourse import bacc

    dt = mybir.dt
    AF = mybir.ActivationFunctionType
    ALU = mybir.AluOpType
    f32 = dt.float32
    f32r = dt.float32r  # fp32 bits, PE fast mode (1 cyc/col at N>=256)
    mdt = dt.bfloat16 if mult_bf16 else f32
    pdt = dt.bfloat16 if mult_bf16 else f32r  # dtype of P / A (reduce mm operands)

    assert t_steps % 4 == 0
    nc = bacc.Bacc("TRN2", target_bir_lowering=False, debug=False)

    # ---- DRAM I/O ----
    u_d = nc.dram_tensor("u", [t_steps, C, BS], mdt, kind="ExternalInput")
    f0_d = nc.dram_tensor("f0t", [F, BS], f32r, kind="ExternalInput")
    w1t_d = nc.dram_tensor("w1t", [H, W], f32r, kind="ExternalInput")
    w2t_d = nc.dram_tensor("w2t", [W, W], f32r, kind="ExternalInput")
    w3t_d = nc.dram_tensor("w3t", [W, HC], f32r, kind="ExternalInput")
    b1_d = nc.dram_tensor("b1c", [W, 1], f32, kind="ExternalInput")
    b2_d = nc.dram_tensor("b2c", [W, 1], f32, kind="ExternalInput")
    b3_d = nc.dram_tensor("b3r", [NCHUNK, HC], f32r, kind="ExternalInput")
    a_d = nc.dram_tensor("ared", [CH, NCHUNK * H], pdt, kind="ExternalInput")
    wpt_d = nc.dram_tensor("wpt", [F, H], f32r, kind="ExternalInput")
    bp_d = nc.dram_tensor("bpr", [1, H], f32r, kind="ExternalInput")
    wr1t_d = nc.dram_tensor("wr1t", [H, R1], f32r, kind="ExternalInput")
    br1_d = nc.dram_tensor("br1c", [R1, 1], f32, kind="ExternalInput")
    wr2t_d = nc.dram_tensor("wr2t", [R1, 1], f32r, kind="ExternalInput")
    br2_d = nc.dram_tensor("br2c", [4, 1], f32, kind="ExternalInput")
    pred_d = nc.dram_tensor("pred", [t_steps, BS], f32, kind="ExternalOutput")

    with tile.TileContext(nc) as tc, ExitStack() as ctx:
        const = ctx.enter_context(tc.tile_pool(name="const", bufs=1))

        def load_const(dram, shape, dtype, tag):
            t_ = const.tile(shape, dtype, tag=tag)
            nc.sync.dma_start(t_[:], dram.ap())
            return t_

        w1t_s = load_const(w1t_d, [H, W], f32r, "w1t")
        w2t_s = load_const(w2t_d, [W, W], f32r, "w2t")
        w3t_s = load_const(w3t_d, [W, HC], f32r, "w3t")
        b1_s = load_const(b1_d, [W, 1], f32, "b1")
        b2_s = load_const(b2_d, [W, 1], f32, "b2")
        a_s = load_const(a_d, [CH, NCHUNK * H], pdt, "ared")
        wpt_s = load_const(wpt_d, [F, H], f32r, "wpt")
        bp_s = load_const(bp_d, [1, H], f32r, "bp")
        wr1t_s = load_const(wr1t_d, [H, R1], f32r, "wr1t")
        br1_s = load_const(br1_d, [R1, 1], f32, "br1")
        wr2t_s = load_const(wr2t_d, [R1, 1], f32r, "wr2t")
        br2_s = load_const(br2_d, [4, 1], f32, "br2")
        f0_s = load_const(f0_d, [F, BS], f32r, "f0")
        # b3 replicated to partitions {0,32,64,96} for row-tiled rank-1 bias mms
        b3_s = const.tile([97, HC], f32r, tag="b3")
        b3_pitch = b3_s[:].ap[0][0]
        nc.sync.dma_start(
            bass.AP(b3_s.tensor, b3_s.offset, [[32 * b3_pitch, NCHUNK], [1, HC]]),
            b3_d.ap(),
        )
        # all-ones rows on partitions 0..96 (only 0,32,64,96 used)
        ones_s = const.tile([97, NB], f32r, tag="ones")
        nc.vector.memset(ones_s[:], 1.0)

        state = ctx.enter_context(tc.tile_pool(name="state", bufs=1))
        h_s = [
            state.tile([H, NB], f32r, tag=f"h{s}", name=f"h{s}") for s in range(NS)
        ]

        u_pool = ctx.enter_context(tc.tile_pool(name="upool", bufs=3))
        x_pool = ctx.enter_context(tc.tile_pool(name="xpool", bufs=2))
        m_pool = ctx.enter_context(tc.tile_pool(name="mpool", bufs=2))
        r_pool = ctx.enter_context(tc.tile_pool(name="rpool", bufs=2))
        pred_pool = ctx.enter_context(tc.tile_pool(name="predpool", bufs=2))

        ps_y = [
            ctx.enter_context(tc.tile_pool(name=f"psy{s}", bufs=1, space="PSUM"))
            for s in range(NS)
        ]
        ps_3 = [
            ctx.enter_context(tc.tile_pool(name=f"ps3{s}", bufs=1, space="PSUM"))
            for s in range(NS)
        ]
        ps_r2 = [
            ctx.enter_context(tc.tile_pool(name=f"psr2{s}", bufs=1, space="PSUM"))
            for s in range(NS)
        ]
        r2acc = [
            ps_r2[s].tile([EP, NB], f32, tag=f"r2acc{s}", name=f"r2acc{s}")
            for s in range(NS)
        ]

        # ---- h0 = Wp @ f0 + bp ----
        for s in range(NS):
            hp = ps_y[s].tile([H, NB], f32, tag=f"y{s}")
            nc.tensor.matmul(
                hp[:], bp_s[:1, :], ones_s[:1, :], start=True, stop=False
            )
            nc.tensor.matmul(
                hp[:],
                wpt_s[:],
                f0_s[:, s * NB : (s + 1) * NB],
                start=False,
                stop=True,
            )
            nc.vector.tensor_copy(h_s[s][:], hp[:])

        # ---- main scan ----
        # pred staging: partition = t % 4, columns = (t//4)*BS + b
        n_g = t_steps // 4
        pred_sb = pred_pool.tile([4, n_g * BS], f32, tag="pred", name="pred_sb")

        for t in range(t_steps):
            g = t // 4
            for s in range(NS):
                cs = slice(s * NB, (s + 1) * NB)

                u_t = u_pool.tile([CH, NB], mdt, tag=f"u{s}")
                nc.sync.dma_start(
                    u_t[:],
                    bass.AP(u_d, (t * C) * BS + s * NB, [[0, CH // C], [BS, C], [1, NB]]),
                )

                # x1 = gelu(W1 @ h + b1)
                y1 = ps_y[s].tile([W, NB], f32, tag=f"y{s}")
                nc.tensor.matmul(y1[:], w1t_s[:], h_s[s][:], start=True, stop=True)
                x1 = x_pool.tile([W, NB], f32r, tag=f"x1{s}")
                nc.scalar.activation(x1[:], y1[:], AF.Gelu_apprx_tanh, bias=b1_s[:])

                # x2 = gelu(W2 @ x1 + b2)
                y2 = ps_y[s].tile([W, NB], f32, tag=f"y{s}")
                nc.tensor.matmul(y2[:], w2t_s[:], x1[:], start=True, stop=True)
                x2 = x_pool.tile([W, NB], f32r, tag=f"x2{s}")
                nc.scalar.activation(x2[:], y2[:], AF.Gelu_apprx_tanh, bias=b2_s[:])

                # Y3 = W3 @ x2 + b3   (4 chunks of 112 rows into one 2-bank psum)
                # One accumulation group per 2KB bank (= two NB-col slices):
                # rank-1 bias mms first (start marks the bank), then W3 chunks.
                y3 = ps_3[s].tile([CH, NCHUNK * NB], f32, tag=f"y3{s}")
                for bank in range(NCHUNK // 2):
                    j0, j1 = 2 * bank, 2 * bank + 1
                    for idx, j in enumerate((j0, j1)):
                        nc.tensor.matmul(
                            y3[:, j * NB : (j + 1) * NB],
                            b3_s[32 * j : 32 * j + 1, j * CH : (j + 1) * CH],
                            ones_s[32 * j : 32 * j + 1, :],
                            start=(idx == 0),
                            stop=False,
                            tile_position=(32 * j, 0),
                        )
                    for idx, j in enumerate((j0, j1)):
                        nc.tensor.matmul(
                            y3[:, j * NB : (j + 1) * NB],
                            w3t_s[:, j * CH : (j + 1) * CH],
                            x2[:],
                            start=False,
                            stop=(idx == 1),
                        )

                # M = tanh(Y3); P = M * U_bcast
                m_t = m_pool.tile([CH, NCHUNK * NB], mdt, tag=f"m{s}")
                nc.scalar.activation(m_t[:], y3[:], AF.Tanh)
                p_t = m_pool.tile([CH, NCHUNK * NB], pdt, tag=f"p{s}")
                m3 = m_t[:].rearrange("p (j n) -> p j n", j=NCHUNK)
                p3 = p_t[:].rearrange("p (j n) -> p j n", j=NCHUNK)
                u3 = bass.AP(
                    u_t.tensor,
                    u_t.offset,
                    [list(u_t.ap[0]), [0, NCHUNK], [1, NB]],
                )
                nc.vector.tensor_tensor(p3, m3, u3, op=ALU.mult)

                # dh = A^T @ P  (c-contraction), accumulate 4 chunks
                dh = ps_y[s].tile([H, NB], f32, tag=f"y{s}")
                for j in range(NCHUNK):
                    nc.tensor.matmul(
                        dh[:],
                        a_s[:, j * H : (j + 1) * H],
                        p_t[:, j * NB : (j + 1) * NB],
                        start=(j == 0),
                        stop=(j == NCHUNK - 1),
                    )

                # h += dh
                nc.vector.tensor_add(h_s[s][:], h_s[s][:], dh[:])

                # readout: r1 = Wr1 @ h; rl = relu(r1 + br1); r2 = Wr2 @ rl
                r1 = ps_y[s].tile([R1, NB], f32, tag=f"y{s}")
                nc.tensor.matmul(r1[:], wr1t_s[:], h_s[s][:], start=True, stop=True)
                rl = r_pool.tile([R1, NB], f32r, tag=f"rl{s}")
                nc.vector.tensor_scalar(
                    rl[:], r1[:], br1_s[:], 0.0, op0=ALU.add, op1=ALU.max
                )
                q = t % 4
                nc.tensor.matmul(
                    r2acc[s][32 * q : 32 * q + 1, :],
                    wr2t_s[:],
                    rl[:],
                    start=True,
                    stop=True,
                    tile_position=(0, 32 * q),
                )
                if q == 3:
                    # evacuate 4 rows of r2acc (partitions 0,32,64,96) + br2
                    src = bass.AP(
                        r2acc[s].tensor,
                        r2acc[s].offset,
                        [[32 * r2acc[s].ap[0][0], 4], [1, NB]],
                    )
                    dst = pred_sb[0:4, g * BS + s * NB : g * BS + (s + 1) * NB]
                    nc.vector.tensor_scalar(dst, src, br2_s[:], None, op0=ALU.add)

        # final: pred_sb (4, G*BS) -> pred_d (T, BS), t = 4g + p
        nc.sync.dma_start(
            bass.AP(pred_d, 0, [[BS, 4], [4 * BS, n_g], [1, BS]]),
            pred_sb[:].rearrange("p (g b) -> p g b", b=BS),
        )

    nc.compile()
    names = [
        "u", "f0t", "w1t", "w2t", "w3t", "b1c", "b2c", "b3r", "ared",
        "wpt", "bpr", "wr1t", "br1c", "wr2t", "br2c",
    ]
    _BUILD_CACHE[key] = (nc, names)
    return nc, names


def _host_prep(time, features, mask, length, Wp, bp, W1, b1, W2, b2, W3, b3,
               Wr1, br1, Wr2, br2, t_steps=T, mult_bf16=False):
    """Shard + marshal inputs into per-core in_maps."""
    time = np.asarray(time, np.float32)
    features = np.asarray(features, np.float32)
    length = np.asarray(length)
    mdt = np.dtype("bfloat16") if mult_bf16 else np.float32
    if mult_bf16:
        import ml_dtypes
        mdt = ml_dtypes.bfloat16

    cp = np.concatenate([time[..., None], features], axis=-1)  # (B, Tfull, C)
    cp_next = np.concatenate([cp[:, 1:], cp[:, -1:]], axis=1)
    active = (np.arange(cp.shape[1])[None, :] < np.asarray(length)[:, None])
    u_full = (cp_next - cp) * active[..., None].astype(np.float32)  # (B, Tfull, C)
    u_full = u_full[:, :t_steps]

    # reduce pattern A: (112, 4*64), A[p, j*64+h] = 1 iff (112*j+p)//7 == h
    a_mat = np.zeros((CH, NCHUNK * H), np.float32)
    for j in range(NCHUNK):
        for p in range(CH):
            h = (CH * j + p) // C
            a_mat[p, j * H + h] = 1.0

    shared = {
        "w1t": np.ascontiguousarray(W1.T, np.float32),          # (64,128)
        "w2t": np.ascontiguousarray(W2.T, np.float32),          # (128,128)
        "w3t": np.ascontiguousarray(W3.T, np.float32),          # (128,448)
        "b1c": np.ascontiguousarray(b1.reshape(W, 1), np.float32),
        "b2c": np.ascontiguousarray(b2.reshape(W, 1), np.float32),
        "b3r": np.broadcast_to(
            np.asarray(b3, np.float32).reshape(1, HC), (NCHUNK, HC)
        ).copy(),                                               # (4,448) replicated
        "ared": a_mat.astype(mdt),
        "wpt": np.ascontiguousarray(Wp.T, np.float32),          # (6,64)
        "bpr": np.ascontiguousarray(bp.reshape(1, H), np.float32),
        "wr1t": np.ascontiguousarray(Wr1.T, np.float32),        # (64,32)
        "br1c": np.ascontiguousarray(br1.reshape(R1, 1), np.float32),
        "wr2t": np.ascontiguousarray(Wr2.T, np.float32),        # (32,1)
        "br2c": np.full((4, 1), np.float32(br2.reshape(-1)[0]), np.float32),
    }
    in_maps = []
    for i in range(NCORES):
        bsl = slice(i * BS, (i + 1) * BS)
        u_core = np.ascontiguousarray(
            u_full[bsl].transpose(1, 2, 0)
        ).astype(mdt)                                            # (T, C, BS)
        f0t = np.ascontiguousarray(features[bsl, 0, :].T, np.float32)  # (6, BS)
        m = dict(shared)
        m["u"] = u_core
        m["f0t"] = f0t
        in_maps.append(m)
    return in_maps


def kernel(**inputs):
    from concourse.bass_utils import run_bass_kernel_spmd

    t_steps = T
    mult_bf16 = False
    nc, _ = _build(t_steps=t_steps, mult_bf16=mult_bf16)
    in_maps = _host_prep(**inputs, t_steps=t_steps, mult_bf16=mult_bf16)
    res = run_bass_kernel_spmd(nc, in_maps, list(range(NCORES)))
    preds = [res.results[i]["pred"] for i in range(NCORES)]  # (T, BS) each
    out = np.concatenate([p.T for p in preds], axis=0)  # (B, T)
    return np.ascontiguousarray(out.astype(np.float32))
